# revision 1
# baseline (speedup 1.0000x reference)
"""Trainium2 Bass kernel for nn_MemoryBlock (scatter_memory).

Mathematical identity: softmax over the memory-unit axis U produces rows
that sum to exactly 1, so

    out[b] = relu( mean_u( sum_n attn[b,n,u] * V[b,n,:] ) @ Wo + bo )
           = relu( ((sum_n X[b,n,:]) @ Wv + N*bv) / U @ Wo + bo )

-- the whole K/scores/softmax path cancels algebraically, leaving a
memory-bound column-sum of X (reading 134 MB is the roofline) plus two
tiny matmuls.

Sharding: data-parallel over batch B=16 across 8 cores (2 batches/core),
small weights replicated (host pre-folds Wv/U and bv*N/U).

Raw Bass (no TileContext): explicit per-engine programs and semaphores
for a minimal start/stop bracket.

Per core (2 batches): SP issues the 16x 1MB X-chunk DMAs immediately;
ACT ring carries the small constant DMAs; PE runs the fp32r ones-matmul
column-sum per chunk accumulating in PSUM, then folds/transposes; DVE/ACT
do the tiny finale; teardown is one wait on the output DMA + sem clears.
"""

import contextlib

import numpy as np

B, N, FEAT, MEM, U = 16, 8192, 256, 128, 512
NCORES = 8
BPC = B // NCORES

CH = 8              # rows-per-partition per DMA chunk -> [128, CH*FEAT] = 1 MB
RPP = N // 128      # 64 rows per partition per batch
NCH = RPP // CH     # 8 chunks per batch
MMW = 512           # matmul moving free width (one PSUM bank of fp32)
MM_PER_CHUNK = CH * FEAT // MMW  # 4

_built = None


def _ensure_axon_hooks():
    try:
        import antenv.axon_hooks  # noqa: F401
        return
    except ImportError:
        pass
    import sys
    import types

    m = types.ModuleType("antenv.axon_hooks")
    holder = [None]
    m.set_axon_ntff_profile_hook = lambda h: holder.__setitem__(0, h)
    m.get_axon_ntff_profile_hook = lambda: holder[0]
    sys.modules["antenv.axon_hooks"] = m
    try:
        import antenv

        antenv.axon_hooks = m
    except ImportError:
        pass


def _build():
    import concourse.bacc as bacc
    import concourse.mybir as mybir

    f32 = mybir.dt.float32
    f32r = mybir.dt.float32r
    AF = mybir.ActivationFunctionType
    nc = bacc.Bacc(None, enable_partition_id=False, monotonic_sem_count=0)

    X_d = nc.dram_tensor("Xs", [BPC, N, FEAT], f32r, kind="ExternalInput")
    Wv_d = nc.dram_tensor("Wvs", [2, 128, MEM], f32, kind="ExternalInput")
    Wo_d = nc.dram_tensor("Wos", [MEM, MEM], f32, kind="ExternalInput")
    # biases packed+padded to 512B/partition rows so the DMA uses
    # line-rate descriptors instead of 4-byte packets: col0=bv', col1=bo
    bias_d = nc.dram_tensor("biasc", [MEM, 128], f32, kind="ExternalInput")
    ones_d = nc.dram_tensor("onesc", [128, 128], f32r, kind="ExternalInput")
    out_d = nc.dram_tensor("outT", [MEM, BPC], f32, kind="ExternalOutput")

    ctx = contextlib.ExitStack()
    with ctx:
        xts = [
            ctx.enter_context(
                nc.sbuf_tensor(f"xt{i}", [128, CH * FEAT], f32r)
            )
            for i in range(BPC * NCH)
        ]
        ones = ctx.enter_context(nc.sbuf_tensor("ones", [128, 128], f32r))
        one_f = ctx.enter_context(nc.sbuf_tensor("one_f", [1, 1], f32))
        wv_sb = ctx.enter_context(nc.sbuf_tensor("wv_sb", [128, 2 * MEM], f32))
        wo_sb = ctx.enter_context(nc.sbuf_tensor("wo_sb", [128, MEM], f32))
        bias_sb = ctx.enter_context(nc.sbuf_tensor("bias_sb", [128, 128], f32))
        stq = ctx.enter_context(nc.sbuf_tensor("stq", [128, 2 * BPC], f32))
        srows = [
            ctx.enter_context(nc.sbuf_tensor(f"srow{i}", [1, MMW], f32))
            for i in range(BPC)
        ]
        out0 = ctx.enter_context(nc.sbuf_tensor("out0", [128, BPC], f32))
        res = ctx.enter_context(nc.sbuf_tensor("res", [128, BPC], f32))

        pss = [
            ctx.enter_context(nc.psum_tensor(f"ps{i}", [1, MMW], f32))
            for i in range(BPC)
        ]
        pts = [
            ctx.enter_context(nc.psum_tensor(f"pt{i}", [128, BPC], f32))
            for i in range(BPC)
        ]
        psv = ctx.enter_context(nc.psum_tensor("psv", [128, BPC], f32))
        pso = ctx.enter_context(nc.psum_tensor("pso", [128, BPC], f32))

        dsems = [
            ctx.enter_context(nc.semaphore(f"dsem{i}"))   # one per X chunk
            for i in range(BPC * NCH)
        ]
        csem = ctx.enter_context(nc.semaphore("csem"))    # const DMAs
        onesem = ctx.enter_context(nc.semaphore("onesem"))  # ones DMA
        osem = ctx.enter_context(nc.semaphore("osem"))    # output DMA
        pesem = ctx.enter_context(nc.semaphore("pesem"))  # PE milestones
        asem = ctx.enter_context(nc.semaphore("asem"))    # ACT milestones
        vsem = ctx.enter_context(nc.semaphore("vsem"))    # DVE milestones
        sem_nums = sorted(
            s.num for s in (*dsems, csem, onesem, osem, pesem, asem, vsem)
        )

        with nc.Block() as block:

            @block.sync
            def _(sync):
                # X chunk DMAs immediately, in consumption order (FIFO ring
                # -> in-order completion -> dsem thresholds are per-chunk)
                for b in range(BPC):
                    Xb = X_d[b].rearrange("(p r) f -> p (r f)", p=128)
                    for c in range(NCH):
                        sync.dma_start(
                            out=xts[b * NCH + c][:, :],
                            in_=Xb[:, c * CH * FEAT : (c + 1) * CH * FEAT],
                        ).then_inc(dsems[b * NCH + c], 16)
                # output DMA after the finale
                sync.wait_ge(asem, BPC + 1)
                sync.dma_start(out=out_d[:, :], in_=res[:, :]).then_inc(osem, 16)

            @block.scalar
            def _(scalar):
                # consts on the ACT HWDGE ring: ones first (own sem so the
                # colsum is gated only on it), then wv0, wv1, wo, biases
                scalar.dma_start(out=ones[:, :], in_=ones_d[:, :]).then_inc(onesem, 16)
                scalar.dma_start(out=wv_sb[:, 0:MEM], in_=Wv_d[0]).then_inc(csem, 16)
                scalar.dma_start(out=wv_sb[:, MEM : 2 * MEM], in_=Wv_d[1]).then_inc(
                    csem, 16
                )
                scalar.dma_start(out=wo_sb[:, :], in_=Wo_d[:, :]).then_inc(csem, 16)
                scalar.dma_start(out=bias_sb[:, :], in_=bias_d[:, :]).then_inc(csem, 16)
                # per-batch psum row -> SBUF copy (transpose lhsT must be SBUF)
                for b in range(BPC):
                    scalar.wait_ge(pesem, b + 1)
                    nc.scalar.activation(
                        out=srows[b][:, :],
                        in_=pss[b][0:1, :],
                        func=AF.Copy,
                        scale=1.0,
                    ).then_inc(asem, 1)
                # final relu
                scalar.wait_ge(pesem, BPC + 4)
                scalar.wait_ge(csem, 64)
                nc.scalar.activation(
                    out=res[:, :],
                    in_=pso[:, :],
                    func=AF.Relu,
                    bias=bias_sb[:, 1:2],
                    scale=1.0,
                ).then_inc(asem, 1)

            @block.tensor
            def _(pe):
                pe.wait_ge(onesem, 16)
                for b in range(BPC):
                    k = 0
                    nmm = NCH * MM_PER_CHUNK
                    for c in range(NCH):
                        pe.wait_ge(dsems[b * NCH + c], 16)
                        for m in range(MM_PER_CHUNK):
                            ins = nc.tensor.matmul(
                                pss[b][:, :],
                                lhsT=ones[:, 0:1],
                                rhs=xts[b * NCH + c][:, m * MMW : (m + 1) * MMW],
                                start=(k == 0),
                                stop=(k == nmm - 1),
                            )
                            k += 1
                    ins.then_inc(pesem, 1)  # pesem: b+1 after batch b colsum
                # fold even/odd + transpose, per batch, via accumulating
                # PE transposes reading srows
                pe.wait_ge(vsem, 1)  # one_f memset
                for b in range(BPC):
                    pe.wait_ge(asem, b + 1)
                    last = None
                    for h in range(2):
                        nc.tensor.matmul(
                            pts[b][:, h : h + 1],
                            lhsT=srows[b][0:1, h * 128 : (h + 1) * 128],
                            rhs=one_f[0:1, 0:1],
                            is_transpose=True,
                            start=True,
                            stop=False,
                        )
                        last = nc.tensor.matmul(
                            pts[b][:, h : h + 1],
                            lhsT=srows[b][0:1, FEAT + h * 128 : FEAT + (h + 1) * 128],
                            rhs=one_f[0:1, 0:1],
                            is_transpose=True,
                            start=False,
                            stop=True,
                        )
                    last.then_inc(pesem, 1)  # pesem: BPC+1+b
                # psv = Wv'.T @ stq
                pe.wait_ge(csem, 64)
                pe.wait_ge(vsem, 1 + 2 * BPC)
                nc.tensor.matmul(
                    psv[:, :], lhsT=wv_sb[:, 0:MEM], rhs=stq[:, 0:BPC],
                    start=True, stop=False,
                )
                nc.tensor.matmul(
                    psv[:, :], lhsT=wv_sb[:, MEM : 2 * MEM], rhs=stq[:, BPC : 2 * BPC],
                    start=False, stop=True,
                ).then_inc(pesem, 1)  # pesem: BPC+3
                pe.wait_ge(vsem, 2 + 2 * BPC)  # out0 ready
                nc.tensor.matmul(
                    pso[:, :], lhsT=wo_sb[:, :], rhs=out0[:, :], start=True, stop=True
                ).then_inc(pesem, 1)  # pesem: BPC+4

            @block.vector
            def _(vector):
                nc.vector.memset(one_f[:, :], 1.0).then_inc(vsem, 1)
                # stq columns h-major: (h0b0, h0b1, h1b0, h1b1)
                for b in range(BPC):
                    vector.wait_ge(pesem, BPC + 1 + b)
                    nc.vector.tensor_copy(
                        out=stq[:, b : b + 1], in_=pts[b][:, 0:1]
                    ).then_inc(vsem, 1)
                    nc.vector.tensor_copy(
                        out=stq[:, BPC + b : BPC + b + 1], in_=pts[b][:, 1:2]
                    ).then_inc(vsem, 1)
                vector.wait_ge(pesem, BPC + 3)
                vector.wait_ge(csem, 64)
                nc.vector.tensor_scalar_add(
                    out=out0[:, :], in0=psv[:, :], scalar1=bias_sb[:, 0:1]
                ).then_inc(vsem, 1)

            @block.gpsimd
            def _(gpsimd):
                gpsimd.wait_ge(osem, 16)

            # all-engine sync, then zero the sems so a re-execution of the
            # loaded NEFF starts clean
            nc.all_engine_barrier()
            nc.gpsimd.sem_clear(range(sem_nums[0], sem_nums[-1] + 1))

    if not nc.is_finalized():
        nc.finalize()
    return nc


def kernel(X, mem, Wk, bk, Wv, bv, Wo, bo):
    global _built
    _ensure_axon_hooks()
    from concourse.bass_utils import run_bass_kernel_spmd

    if _built is None:
        _built = _build()
    nc = _built

    X = np.asarray(X, dtype=np.float32)
    Wvs = np.ascontiguousarray(
        (np.asarray(Wv, dtype=np.float32) / float(U)).reshape(2, 128, MEM)
    )
    Wos = np.ascontiguousarray(np.asarray(Wo, dtype=np.float32))
    biasc = np.zeros((MEM, 128), dtype=np.float32)
    biasc[:, 0] = np.asarray(bv, dtype=np.float32) * (N / float(U))
    biasc[:, 1] = np.asarray(bo, dtype=np.float32)
    onesc = np.ones((128, 128), dtype=np.float32)

    in_maps = [
        {
            "Xs": np.ascontiguousarray(X[i * BPC : (i + 1) * BPC]),
            "Wvs": Wvs,
            "Wos": Wos,
            "biasc": biasc,
            "onesc": onesc,
        }
        for i in range(NCORES)
    ]
    r = run_bass_kernel_spmd(nc, in_maps, list(range(NCORES)))
    kernel._last_results = r

    out = np.empty((B, MEM), dtype=np.float32)
    for i in range(NCORES):
        out[i * BPC : (i + 1) * BPC] = r.results[i]["outT"].T
    return out



# revision 17
# speedup vs baseline: 1.0132x; 1.0132x over previous
"""Trainium2 Bass kernel for nn_MemoryBlock (scatter_memory).

Mathematical identity: softmax over the memory-unit axis U produces rows
that sum to exactly 1, so the attention path cancels, and the two Linear
layers fold into one (no nonlinearity between them):

    out[b] = relu( s_b @ (Wv Wo)/U + (N/U) bv Wo + bo ),   s_b = sum_n X[b,n,:]

-- a memory-bound column-sum of X (reading 134 MB is the roofline) plus one
tiny 256x128 matmul, with Wvo = (Wv Wo)/U folded on the host.

Sharding: data-parallel over batch B=16 across 8 cores (2 batches/core).

Raw Bass (no TileContext): explicit per-engine programs and semaphores.
Per core (2 batches): SP ring issues the X-chunk DMAs immediately; ACT ring
carries the small constant DMAs and the two per-batch output DMAs; PE runs
the fp32r ones-matmul column-sum per chunk accumulating in PSUM; per batch
the finale is fold(DVE) -> 2 PE transposes -> stq copy(DVE) -> 2 PE matmuls
against Wvo -> DVE bias+relu -> [1,512B] output DMA.  Batch 0's finale
completes in the DMA shadow of batch 1; only batch 1's short chain trails
the last X byte.

Completion fences use dummy DMAs on the same HWDGE ring (per-engine FIFO
ordering) so thresholds are correct regardless of how a <128-partition
DMA distributes its 16 semaphore increments across SDMA engines.
"""

import contextlib

import numpy as np

B, N, FEAT, MEM, U = 16, 8192, 256, 128, 512
NCORES = 8
BPC = B // NCORES

CH = 8              # rows-per-partition per full DMA chunk -> [128, CH*FEAT] = 1 MB
RPP = N // 128      # 64 rows per partition per batch
MMW = 512           # matmul moving free width (one PSUM bank of fp32)
# per batch: 7 full chunks (8 rows) + 2 half chunks (4 rows) so the tail
# colsum after the final DMA is only 2 matmuls
NFULL = 7
NHALF = 2
NDESC = NFULL + NHALF  # X descriptors per batch

_built = None


def _ensure_axon_hooks():
    try:
        import antenv.axon_hooks  # noqa: F401
        return
    except ImportError:
        pass
    import sys
    import types

    m = types.ModuleType("antenv.axon_hooks")
    holder = [None]
    m.set_axon_ntff_profile_hook = lambda h: holder.__setitem__(0, h)
    m.get_axon_ntff_profile_hook = lambda: holder[0]
    sys.modules["antenv.axon_hooks"] = m
    try:
        import antenv

        antenv.axon_hooks = m
    except ImportError:
        pass


def _build():
    import concourse.bacc as bacc
    import concourse.mybir as mybir

    f32 = mybir.dt.float32
    f32r = mybir.dt.float32r
    nc = bacc.Bacc(None, enable_partition_id=False, monotonic_sem_count=0)

    X_d = nc.dram_tensor("Xs", [BPC, N, FEAT], f32r, kind="ExternalInput")
    # host-swizzled: wvo[f, h*128+j] = ((Wv@Wo)/U)[h*128+f, j]
    Wvo_d = nc.dram_tensor("Wvoc", [MEM, 2 * MEM], f32, kind="ExternalInput")
    bvo_d = nc.dram_tensor("bvoc", [BPC, MEM], f32, kind="ExternalInput")
    ones_d = nc.dram_tensor("onesc", [128, 1], f32r, kind="ExternalInput")
    out_d = nc.dram_tensor("outb", [BPC, MEM], f32, kind="ExternalOutput")
    scr_d = nc.dram_tensor("scr", [128, 16], f32, kind="Internal")

    ctx = contextlib.ExitStack()
    with ctx:
        # per-batch chunk tiles: 7 full [128, 2048] + 2 half [128, 1024]
        xts = []
        for b in range(BPC):
            tiles = [
                ctx.enter_context(
                    nc.sbuf_tensor(f"xt{b}_{c}", [128, CH * FEAT], f32r)
                )
                for c in range(NFULL)
            ] + [
                ctx.enter_context(
                    nc.sbuf_tensor(f"xh{b}_{h}", [128, CH * FEAT // 2], f32r)
                )
                for h in range(NHALF)
            ]
            xts.append(tiles)
        ones = ctx.enter_context(nc.sbuf_tensor("ones", [128, 1], f32r))
        one_f = ctx.enter_context(nc.sbuf_tensor("one_f", [1, 1], f32))
        wvo_sb = ctx.enter_context(nc.sbuf_tensor("wvo_sb", [128, 2 * MEM], f32))
        bvo_sb = ctx.enter_context(nc.sbuf_tensor("bvo_sb", [33, MEM], f32))
        sraw = ctx.enter_context(nc.sbuf_tensor("sraw", [1, BPC * MMW], f32))
        srow = ctx.enter_context(nc.sbuf_tensor("srow", [1, BPC * 256], f32))
        stq = ctx.enter_context(nc.sbuf_tensor("stq", [128, 2 * BPC], f32))
        res = ctx.enter_context(nc.sbuf_tensor("res", [33, MEM], f32))
        probe_sb = ctx.enter_context(nc.sbuf_tensor("probe_sb", [128, 64], f32))

        pss = [
            ctx.enter_context(nc.psum_tensor(f"ps{b}", [1, MMW], f32))
            for b in range(BPC)
        ]
        pts = ctx.enter_context(nc.psum_tensor("pts", [128, 2 * BPC], f32))
        # matmul out base partition must be 0/32/64: batch b's result row
        # lives at partition 32*b
        pres = ctx.enter_context(nc.psum_tensor("pres", [33, MEM], f32))

        dsems = [
            ctx.enter_context(nc.semaphore(f"dsem{i}"))
            for i in range(BPC * NDESC)
        ]
        osem = ctx.enter_context(nc.semaphore("osem"))      # output DMAs
        csem = ctx.enter_context(nc.semaphore("csem"))      # const DMAs (trace)
        pesem = ctx.enter_context(nc.semaphore("pesem"))    # PE milestones
        vsem = ctx.enter_context(nc.semaphore("vsem"))      # DVE milestones
        onesem = ctx.enter_context(nc.semaphore("onesem"))  # ones DMA
        dum1 = ctx.enter_context(nc.semaphore("dum1"))      # consts fence
        dum2 = ctx.enter_context(nc.semaphore("dum2"))      # outputs fence
        psems = [
            ctx.enter_context(nc.semaphore(f"psem{i}"))     # DMA-split probes
            for i in range(4)
        ]

        with nc.Block() as block:

            @block.sync
            def _(sync):
                # X chunk DMAs immediately, in consumption order (FIFO ring
                # -> in-order completion)
                for b in range(BPC):
                    Xb = X_d[b].rearrange("(p r) f -> p (r f)", p=128)
                    for c in range(NFULL):
                        sync.dma_start(
                            out=xts[b][c][:, :],
                            in_=Xb[:, c * CH * FEAT : (c + 1) * CH * FEAT],
                        ).then_inc(dsems[b * NDESC + c], 16)
                    half = CH * FEAT // 2
                    for h in range(NHALF):
                        off = NFULL * CH * FEAT + h * half
                        sync.dma_start(
                            out=xts[b][NFULL + h][:, :],
                            in_=Xb[:, off : off + half],
                        ).then_inc(dsems[b * NDESC + NFULL + h], 16)

            @block.scalar
            def _(scalar):
                # consts on the ACT HWDGE ring (csem is trace-visibility only)
                # ones first: it gates the first colsum matmul; 128-partition
                # so >=16 is a safe wait under either split model
                scalar.dma_start(out=ones[:, :], in_=ones_d[:, :]).then_inc(onesem, 16)
                scalar.dma_start(out=bvo_sb[0:1, :], in_=bvo_d[0:1, :]).then_inc(csem, 16)
                scalar.dma_start(
                    out=bvo_sb[32:33, :], in_=bvo_d[1:2, :]
                ).then_inc(csem, 16)
                scalar.dma_start(out=wvo_sb[:, :], in_=Wvo_d[:, :]).then_inc(csem, 16)
                # split-semantics probes: partition counts 92 / 28 / 2 / 1.
                # Nothing waits on these; the trace shows how many increments
                # each delivers.
                for np_, i in ((92, 0), (28, 1), (2, 2), (1, 3)):
                    scalar.dma_start(
                        out=probe_sb[0:np_, i * 8 : (i + 1) * 8],
                        in_=Wvo_d[0:np_, 0:8],
                    ).then_inc(psems[i], 16)
                # dummy1 fence: 128-partition read => one segment on every
                # SDMA engine, so it delivers exactly 16 increments under
                # either split model; per-engine FIFO makes dum1>=16 imply
                # bvo+wvo landed.
                scalar.dma_start(
                    out=probe_sb[:, 48:64], in_=Wvo_d[:, 0:16]
                ).then_inc(dum1, 16)
                # per-batch output DMAs: row b of res -> out row b (512 B)
                scalar.wait_ge(vsem, 6)
                scalar.dma_start(out=out_d[0:1, :], in_=res[0:1, :]).then_inc(osem, 16)
                scalar.wait_ge(vsem, 10)
                scalar.dma_start(out=out_d[1:2, :], in_=res[32:33, :]).then_inc(osem, 16)
                # dummy2 fence: 128-partition write; its partition-0 segment
                # is FIFO-ordered behind both output writes on the same
                # engine+route, so dum2>=16 implies the outputs are in DRAM.
                scalar.dma_start(out=scr_d[:, :], in_=probe_sb[:, 0:16]).then_inc(
                    dum2, 16
                )

            @block.tensor
            def _(pe):
                pe.wait_ge(onesem, 16)
                for b in range(BPC):
                    # column-sum of batch b via ones-matmuls into pss[b][1,512]
                    k = 0
                    nmm = NFULL * 4 + NHALF * 2
                    for c in range(NFULL):
                        pe.wait_ge(dsems[b * NDESC + c], 16)
                        for m in range(4):
                            ins = nc.tensor.matmul(
                                pss[b][:, :],
                                lhsT=ones[:, 0:1],
                                rhs=xts[b][c][:, m * MMW : (m + 1) * MMW],
                                start=(k == 0),
                                stop=(k == nmm - 1),
                            )
                            k += 1
                    for h in range(NHALF):
                        pe.wait_ge(dsems[b * NDESC + NFULL + h], 16)
                        for m in range(2):
                            ins = nc.tensor.matmul(
                                pss[b][:, :],
                                lhsT=ones[:, 0:1],
                                rhs=xts[b][NFULL + h][:, m * MMW : (m + 1) * MMW],
                                start=(k == 0),
                                stop=(k == nmm - 1),
                            )
                            k += 1
                    ins.then_inc(pesem, 1)  # pesem: 3b+1
                    # transpose folded row (2 x 128) into pts cols 2b+h
                    pe.wait_ge(vsem, 3 if b == 0 else 7)
                    for h in range(2):
                        ins = nc.tensor.matmul(
                            pts[:, 2 * b + h : 2 * b + h + 1],
                            lhsT=srow[
                                0:1, b * 256 + h * 128 : b * 256 + (h + 1) * 128
                            ],
                            rhs=one_f[0:1, 0:1],
                            is_transpose=True,
                            start=True,
                            stop=True,
                        )
                    ins.then_inc(pesem, 1)  # pesem: 3b+2
                    # final matmul: pres[b, :] = sum_h stq[:,2b+h]^T @ Wvo_h
                    pe.wait_ge(vsem, 4 if b == 0 else 8)
                    if b == 0:
                        pe.wait_ge(dum1, 16)
                    for h in range(2):
                        ins = nc.tensor.matmul(
                            pres[32 * b : 32 * b + 1, :],
                            lhsT=stq[:, 2 * b + h : 2 * b + h + 1],
                            rhs=wvo_sb[:, h * MEM : (h + 1) * MEM],
                            start=(h == 0),
                            stop=(h == 1),
                        )
                    ins.then_inc(pesem, 1)  # pesem: 3b+3

            @block.vector
            def _(vector):
                nc.vector.memset(one_f[:, :], 1.0).then_inc(vsem, 2)  # =2
                for b in range(BPC):
                    # colsum row PSUM -> SBUF, then fold even/odd halves
                    # (DVE may read at most one PSUM operand per op)
                    vector.wait_ge(pesem, 3 * b + 1)
                    nc.vector.tensor_copy(
                        out=sraw[0:1, b * MMW : (b + 1) * MMW], in_=pss[b][0:1, :]
                    )
                    nc.vector.tensor_add(
                        out=srow[0:1, b * 256 : (b + 1) * 256],
                        in0=sraw[0:1, b * MMW : b * MMW + 256],
                        in1=sraw[0:1, b * MMW + 256 : (b + 1) * MMW],
                    ).then_inc(vsem, 1)  # =4b+3
                    # transposed cols PSUM -> SBUF for the final matmul lhsT
                    vector.wait_ge(pesem, 3 * b + 2)
                    nc.vector.tensor_copy(
                        out=stq[:, 2 * b : 2 * b + 2],
                        in_=pts[:, 2 * b : 2 * b + 2],
                    ).then_inc(vsem, 1)  # =4b+4
                    # bias + relu on the finished pres row
                    vector.wait_ge(pesem, 3 * b + 3)
                    if b == 0:
                        vector.wait_ge(dum1, 16)
                    nc.vector.tensor_add(
                        out=res[32 * b : 32 * b + 1, :],
                        in0=pres[32 * b : 32 * b + 1, :],
                        in1=bvo_sb[32 * b : 32 * b + 1, :],
                    ).then_inc(vsem, 1)  # =4b+5
                    nc.vector.tensor_scalar_max(
                        out=res[32 * b : 32 * b + 1, :],
                        in0=res[32 * b : 32 * b + 1, :],
                        scalar1=0.0,
                    ).then_inc(vsem, 1)  # =4b+6

            @block.gpsimd
            def _(gpsimd):
                gpsimd.wait_ge(osem, 2)
                gpsimd.wait_ge(dum2, 16)

    if not nc.is_finalized():
        nc.finalize()
    return nc


def kernel(X, mem, Wk, bk, Wv, bv, Wo, bo):
    global _built
    _ensure_axon_hooks()
    from concourse.bass_utils import run_bass_kernel_spmd

    if _built is None:
        _built = _build()
    nc = _built

    X = np.asarray(X, dtype=np.float32)
    Wv64 = np.asarray(Wv, dtype=np.float64)
    Wo64 = np.asarray(Wo, dtype=np.float64)
    Wvo = ((Wv64 @ Wo64) / float(U)).astype(np.float32)          # [FEAT, MEM]
    # wvo_sb[f, h*128+j] = Wvo[h*128+f, j]
    Wvoc = np.ascontiguousarray(
        Wvo.reshape(2, MEM, MEM).transpose(1, 0, 2).reshape(MEM, 2 * MEM)
    )
    bvo = (
        (float(N) / float(U)) * (np.asarray(bv, np.float64) @ Wo64)
        + np.asarray(bo, np.float64)
    ).astype(np.float32)                                          # [MEM]
    bvoc = np.ascontiguousarray(np.broadcast_to(bvo, (BPC, MEM)))

    onesc = np.ones((128, 1), dtype=np.float32)
    in_maps = [
        {
            "Xs": np.ascontiguousarray(X[i * BPC : (i + 1) * BPC]),
            "Wvoc": Wvoc,
            "bvoc": bvoc,
            "onesc": onesc,
        }
        for i in range(NCORES)
    ]
    r = run_bass_kernel_spmd(nc, in_maps, list(range(NCORES)))
    kernel._last_results = r

    out = np.empty((B, MEM), dtype=np.float32)
    for i in range(NCORES):
        out[i * BPC : (i + 1) * BPC] = r.results[i]["outb"]
    return out
